# revision 27
# baseline (speedup 1.0000x reference)

# Trainium2 Bass kernel for nn_CameraAwareLoss (self-contained).
#
# Strategy (8 NeuronCores, data-parallel over groups):
#   - 16384 rows = 4096 groups x 4 samples, d=1024. Each core owns 512
#     groups. Group-per-partition layout: x loaded as [128, 4*1024] tiles
#     (partition = group, free = 4 sample rows), so group sums are cheap
#     slice adds and the fp8 payload is already in gather layout.
#   - Centers first: cen = sum of 4 raw rows, l2-normalize, transpose on
#     PE -> fcT8 (fp8 x16) -> pay_fc -> AllGather #1 (0.5MB/rank).
#   - Row norms + fp8(x16) rows -> pay_fa -> AllGather #2 (2MB/rank).
#     A dummy write of fcT8 bytes into pay_fa's pad row makes the fa
#     gather schedule strictly after the centers ship (keeps AG order).
#   - dist matmul fp8 DoubleRow + same-label mask into bf16 sim; argmin
#     via max8/find_index8 — all overlapping AllGather #2.
#   - m_a (local cross-camera mean) from the fp8 rows via pair dots in
#     the same overlap window. No third collective: m_b is recomputed
#     from the gathered hard-negative rows.
#   - Tail: indirect-DMA gather of B rows (cast to bf16 in the DMA),
#     16 cross dots + 6 B-pair dots, m1/m2 combine, loss accumulated on
#     PE. Per-core partial loss summed on host.
import numpy as np
import ml_dtypes

import concourse.bass as bass
import concourse.mybir as mybir
import concourse.bacc as bacc
from concourse import tile
from concourse.bass_utils import run_bass_kernel_spmd

NCORES = 8
NG = 4096          # total groups
G = NG // NCORES   # groups per core (512)
D = 1024
R = G * 4          # rows per core (2048)
PT = G // 128      # group tiles per core (4)
KT = D // 128      # contraction tiles (8)
BIG = 1e6
MARGIN = 0.3
SCL = 16.0         # fp8 pre-quantization scale
IV2 = 1.0 / (SCL * SCL)
PR = R + 4         # payload rows (one pad row for the ordering dep)

f32 = mybir.dt.float32
bf16 = mybir.dt.bfloat16
f8 = mybir.dt.float8e4
u16 = mybir.dt.uint16
u32 = mybir.dt.uint32

_CACHE = {}

AF = mybir.ActivationFunctionType
OP = mybir.AluOpType
PM = mybir.MatmulPerfMode

# fallback switches (flip if a feature misbehaves on hardware)
GB_CAST = True      # indirect DMA casts f8 -> bf16 while gathering B rows
BF16_SIM = True     # accumulate sim in bf16 (drains read PSUM f32)


def _build(variant="full"):
    nc = bacc.Bacc("TRN2", target_bir_lowering=False, debug=False,
                   num_devices=NCORES)

    x_sh = nc.dram_tensor("x_sh", [R, D], f32, kind="ExternalInput")
    lab_bc = nc.dram_tensor("lab_bc", [128, NG], u16, kind="ExternalInput")
    lab_loc = nc.dram_tensor("lab_loc", [G, 1], f32, kind="ExternalInput")
    gtab = nc.dram_tensor("gtab", [NG, 24], f32, kind="ExternalInput")
    atab = nc.dram_tensor("atab", [G, 24], f32, kind="ExternalInput")
    ones_in = nc.dram_tensor("ones1", [128, 1], f32, kind="ExternalInput")
    ident_in = nc.dram_tensor("ident", [128, 128], bf16, kind="ExternalInput")

    loss_out = nc.dram_tensor("loss_part", [1, 1], f32, kind="ExternalOutput")

    fc_full = nc.dram_tensor("fc_full", [NCORES * 128, KT * G], f8,
                             kind="Internal", addr_space="Shared")
    fa_full = nc.dram_tensor("fa_full", [NCORES * PR, D], f8, kind="Internal",
                             addr_space="Shared")

    rg = [list(range(NCORES))]
    SIMDT = bf16 if BF16_SIM else f32

    from contextlib import ExitStack
    with tile.TileContext(nc) as tc:
        with ExitStack() as stack:
            ep = stack.enter_context
            ct = ep(tc.tile_pool(name="consts", bufs=1))
            px = ep(tc.tile_pool(name="px", bufs=2))
            psq = ep(tc.tile_pool(name="psq", bufs=2))
            pfa8 = ep(tc.tile_pool(name="pfa8", bufs=PT))
            pga16 = ep(tc.tile_pool(name="pga16", bufs=2))
            pfc = ep(tc.tile_pool(name="pfc", bufs=2))
            psim = ep(tc.tile_pool(name="psim", bufs=1))
            pgb = ep(tc.tile_pool(name="pgb", bufs=2))
            pscr = ep(tc.tile_pool(name="pscr", bufs=2))
            pprs = ep(tc.tile_pool(name="pprs", bufs=1))
            psmall = ep(tc.tile_pool(name="psmall", bufs=4))
            pnrm = ep(tc.tile_pool(name="pnrm", bufs=6))
            pma = ep(tc.tile_pool(name="pma", bufs=PT))
            ppc = ep(tc.tile_pool(name="ppc", bufs=2, space="PSUM"))
            ptiny = ep(tc.tile_pool(name="ptiny", bufs=2, space="PSUM"))
            pdram = ep(tc.tile_pool(name="pdram", bufs=1, space="DRAM"))
            pay_fc = pdram.tile([128, KT * G], f8, tag="pfcd")
            pay_fa = pdram.tile([PR, D], f8, tag="pfad")

            # ---- constants to SBUF ----
            ones_sb = ct.tile([128, 1], f32, tag="ones")
            nc.sync.dma_start(ones_sb[:], ones_in[:])
            ident_sb = ct.tile([128, 128], bf16, tag="ident")
            nc.sync.dma_start(ident_sb[:], ident_in[:])
            labbc_sb = ct.tile([128, NG], u16, tag="labbc")
            nc.sync.dma_start(labbc_sb[:], lab_bc[:])
            lab_sb = ct.tile([128, PT], f32, tag="labloc")
            atab_sb = ct.tile([128, 24 * PT], f32, tag="atab")
            for p in range(PT):
                nc.sync.dma_start(lab_sb[:, p:p + 1], lab_loc[128 * p:128 * (p + 1), :])
                nc.sync.dma_start(atab_sb[:, 24 * p:24 * (p + 1)],
                                  atab[128 * p:128 * (p + 1), :])

            fcT8 = ct.tile([128, KT, G], f8, tag="fcT8")        # local centers^T
            rhs_all = ct.tile([128, KT, NG], f8, tag="rhsall")  # gathered ^T

            # ---- front: stream x tiles in group-per-partition layout ----
            fa8_tiles = []
            for t in range(PT):
                x_t = px.tile([128, 4 * D], f32, tag="x")
                nc.sync.dma_start(
                    x_t[:],
                    x_sh[512 * t:512 * (t + 1), :].rearrange(
                        "(a x) c -> a (x c)", x=4))
                # center chain (feeds the first AllGather — keep earliest)
                cen = pfc.tile([128, D], f32, tag="cen")
                nc.vector.tensor_tensor(cen[:], x_t[:, 0:D], x_t[:, D:2 * D],
                                        OP.add)
                nc.vector.tensor_tensor(cen[:], cen[:], x_t[:, 2 * D:3 * D],
                                        OP.add)
                nc.vector.tensor_tensor(cen[:], cen[:], x_t[:, 3 * D:4 * D],
                                        OP.add)
                csq = psq.tile([128, D], bf16, tag="csq")
                cn2 = pnrm.tile([128, 1], f32, tag="cn2")
                nc.scalar.activation(csq[:], cen[:], AF.Square, accum_out=cn2[:])
                cnm = pnrm.tile([128, 1], f32, tag="cnm")
                nc.scalar.activation(cnm[:], cn2[:], AF.Sqrt)
                crn = pnrm.tile([128, 1], f32, tag="crn")
                nc.vector.reciprocal(crn[:], cnm[:])
                fc_t = pfc.tile([128, D], bf16, tag="fc")
                nc.vector.tensor_scalar(fc_t[:], cen[:], crn[:], None, OP.mult)
                for kk in range(KT):
                    tp_ps = ptiny.tile([128, 128], bf16, tag="tp")
                    nc.tensor.transpose(tp_ps[:], fc_t[:, 128 * kk:128 * (kk + 1)],
                                        ident_sb[:])
                    nc.vector.tensor_scalar(fcT8[:, kk, 128 * t:128 * (t + 1)],
                                            tp_ps[:], SCL, None, OP.mult)
                # ship this tile's slice of every kk block immediately
                nc.sync.dma_start(
                    pay_fc[:].rearrange("p (k g) -> p k g", k=KT)[
                        :, :, 128 * t:128 * (t + 1)],
                    fcT8[:, :, 128 * t:128 * (t + 1)])
                # row norms + fp8 rows
                sq_t = psq.tile([128, 4 * D], bf16, tag="sq")
                nc.scalar.activation(sq_t[:], x_t[:], AF.Square)
                nm2 = pnrm.tile([128, 4], f32, tag="nm2")
                nc.vector.tensor_reduce(
                    nm2[:].rearrange("p (q o) -> p q o", o=1),
                    sq_t[:].rearrange("p (q d) -> p q d", q=4),
                    mybir.AxisListType.X, OP.add)
                nm = pnrm.tile([128, 4], f32, tag="nm")
                nc.scalar.activation(nm[:], nm2[:], AF.Sqrt)
                rn16 = pnrm.tile([128, 4], f32, tag="rn16")
                nc.vector.reciprocal(rn16[:], nm[:])
                nc.vector.tensor_scalar(rn16[:], rn16[:], SCL, None, OP.mult)
                fa8_t = pfa8.tile([128, 4 * D], f8, tag="fa8")
                for q in range(4):
                    nc.scalar.activation(fa8_t[:, D * q:D * (q + 1)],
                                         x_t[:, D * q:D * (q + 1)], AF.Copy,
                                         scale=rn16[:, q:q + 1])
                nc.sync.dma_start(pay_fa[512 * t:512 * (t + 1), :].rearrange(
                    "(g r) d -> g (r d)", r=4), fa8_t[:])
                fa8_tiles.append(fa8_t)

            nc.gpsimd.collective_compute("AllGather", OP.bypass,
                                         replica_groups=rg,
                                         ins=[pay_fc[:]], outs=[fc_full[:]])
            # ordering dep: pad row written from fcT8 so the fa gather is
            # never scheduled before the centers ship.
            nc.sync.dma_start(pay_fa[R:R + 1, 0:32], fcT8[0:1, KT - 1, 384:416])
            nc.gpsimd.collective_compute("AllGather", OP.bypass,
                                         replica_groups=rg,
                                         ins=[pay_fa[:]], outs=[fa_full[:]])

            # ---- load gathered centers (one DMA per rank) ----
            for r in range(NCORES):
                nc.sync.dma_start(
                    rhs_all[:, :, G * r:G * (r + 1)],
                    fc_full[128 * r:128 * (r + 1), :].rearrange(
                        "p (k g) -> p k g", k=KT))

            # ---- dist matmul (fp8 DoubleRow) + argmin per group-tile ----
            idx_list = []
            for mt in range(PT):
                sim = psim.tile([128, NG], SIMDT, tag="sim")
                nc.vector.tensor_scalar(sim[:], labbc_sb[:],
                                        lab_sb[:, mt:mt + 1], -BIG,
                                        OP.is_equal, OP.mult)
                for ch in range(4):
                    pm_t = ppc.tile([128, 1024], f32, tag="big")
                    for h in range(2):
                        for k2 in range(KT // 2):
                            nc.tensor.matmul(
                                pm_t[:, 512 * h:512 * (h + 1)],
                                lhsT=fcT8[:, 2 * k2:2 * k2 + 2,
                                          128 * mt:128 * (mt + 1)],
                                rhs=rhs_all[:, 2 * k2:2 * k2 + 2,
                                            1024 * ch + 512 * h:
                                            1024 * ch + 512 * (h + 1)],
                                start=(k2 == 0), stop=(k2 == KT // 2 - 1),
                                perf_mode=PM.DoubleRow)
                    nc.vector.tensor_tensor(sim[:, 1024 * ch:1024 * (ch + 1)],
                                            pm_t[:],
                                            sim[:, 1024 * ch:1024 * (ch + 1)],
                                            OP.add)
                mx8 = psmall.tile([128, 8], SIMDT, tag="mx8")
                mi8 = psmall.tile([128, 8], u32, tag="mi8")
                nc.vector.max_with_indices(mx8[:], mi8[:], sim[:])
                idx_list.append(mi8)

            # ---- m_a (local pair dots), in the fa-gather overlap window ----
            ma_list = []
            for mt in range(PT):
                fa8_t = fa8_tiles[mt]
                prs = pprs.tile([128, 6 * D], bf16, tag="prs")
                pos = 0
                for i in range(3):
                    w = 3 - i
                    nc.vector.tensor_tensor(
                        prs[:, D * pos:D * (pos + w)].rearrange(
                            "p (j d) -> p j d", j=w),
                        *bass.broadcast_tensor_aps(
                            fa8_t[:, D * i:D * (i + 1)].rearrange(
                                "p (o d) -> p o d", o=1),
                            fa8_t[:, D * (i + 1):].rearrange(
                                "p (j d) -> p j d", j=w)),
                        OP.mult)
                    pos += w
                d6 = psmall.tile([128, 6], f32, tag="d6")
                for j in range(6):
                    junk = pscr.tile([128, D], bf16, tag="junk")
                    nc.scalar.activation(
                        junk[:], prs[:, D * j:D * (j + 1)], AF.Copy,
                        accum_out=d6[:, j:j + 1])
                # s = sum_k mask6 * d6 ; m_a = s * rec1 * (-2/256)
                sm6 = psmall.tile([128, 6], f32, tag="sm6")
                nc.vector.tensor_tensor(sm6[:], d6[:],
                                        atab_sb[:, 24 * mt + 16:24 * mt + 22],
                                        OP.mult)
                ssum = psmall.tile([128, 1], f32, tag="ssum")
                nc.vector.tensor_reduce(ssum[:], sm6[:], mybir.AxisListType.X,
                                        OP.add)
                ma_t = pma.tile([128, 1], f32, tag="ma")
                nc.vector.tensor_tensor(ma_t[:], ssum[:],
                                        atab_sb[:, 24 * mt + 11:24 * mt + 12],
                                        OP.mult)
                nc.vector.tensor_scalar(ma_t[:], ma_t[:], -2.0 * IV2, None,
                                        OP.mult)
                ma_list.append(ma_t)

            # ---- B-side gathers (need fa_full) ----
            gB_list, btab_list = [], []
            for mt in range(PT):
                idx = idx_list[mt][:, 0:1]
                idxr = psmall.tile([128, 1], u32, tag="idxr")
                nc.vector.tensor_scalar(idxr[:], idx, 9, None,
                                        OP.arith_shift_right)
                nc.vector.tensor_tensor(idxr[:], idxr[:], idx, OP.add)
                gB = pgb.tile([128, 4 * D], bf16 if GB_CAST else f8, tag="gb")
                nc.gpsimd.indirect_dma_start(
                    out=gB[:], out_offset=None,
                    in_=fa_full[:].rearrange("(a x) c -> a (x c)", x=4),
                    in_offset=bass.IndirectOffsetOnAxis(ap=idxr[:], axis=0))
                btab = psmall.tile([128, 24], f32, tag="btab")
                nc.gpsimd.indirect_dma_start(
                    out=btab[:], out_offset=None, in_=gtab[:],
                    in_offset=bass.IndirectOffsetOnAxis(ap=idx, axis=0))
                gB_list.append(gB)
                btab_list.append(btab)

            # ---- per-group stats + loss ----
            ploss = ptiny.tile([1, 1], f32, tag="tiny", name="ploss")
            for mt in range(PT):
                gB = gB_list[mt]
                btab = btab_list[mt]
                ga16 = pga16.tile([128, 4 * D], bf16, tag="ga16")
                nc.vector.tensor_scalar(ga16[:], fa8_tiles[mt][:], 1.0, None,
                                        OP.mult)

                # cross products PAB[p, 4i+j] = A_i . B_j
                # mults: i<3 on DVE, i=3 on gpsimd; reduces: i=0 DVE 3D,
                # i=2 gpsimd-free DVE 3D, i=1,3 scalar accum — balances the
                # three engines in the post-gather phase.
                pab = psmall.tile([128, 16], f32, tag="pab")
                for i in range(4):
                    scr = pscr.tile([128, 4 * D], bf16, tag="scr")
                    scr3 = scr[:].rearrange("p (j d) -> p j d", j=4)
                    a3 = ga16[:, D * i:D * (i + 1)].rearrange(
                        "p (o d) -> p o d", o=1)
                    b3 = gB[:].rearrange("p (j d) -> p j d", j=4)
                    a3b, b3b = bass.broadcast_tensor_aps(a3, b3)
                    meng = nc.gpsimd if i == 3 else nc.vector
                    meng.tensor_tensor(scr3, a3b, b3b, OP.mult)
                    if i % 2 == 0:
                        nc.vector.tensor_reduce(
                            pab[:, 4 * i:4 * (i + 1)].rearrange(
                                "p (j o) -> p j o", o=1),
                            scr3, mybir.AxisListType.X, OP.add)
                    else:
                        for j in range(4):
                            junk = pscr.tile([128, D], bf16, tag="junk")
                            nc.scalar.activation(
                                junk[:], scr[:, D * j:D * (j + 1)], AF.Copy,
                                accum_out=pab[:, 4 * i + j:4 * i + j + 1])

                # B-side pair dots for m_b (mults on gpsimd)
                prsb = pprs.tile([128, 6 * D], bf16, tag="prs")
                pos = 0
                for i in range(3):
                    w = 3 - i
                    nc.gpsimd.tensor_tensor(
                        prsb[:, D * pos:D * (pos + w)].rearrange(
                            "p (j d) -> p j d", j=w),
                        *bass.broadcast_tensor_aps(
                            gB[:, D * i:D * (i + 1)].rearrange(
                                "p (o d) -> p o d", o=1),
                            gB[:, D * (i + 1):].rearrange(
                                "p (j d) -> p j d", j=w)),
                        OP.mult)
                    pos += w
                d6b = psmall.tile([128, 6], f32, tag="d6b")
                for j in range(6):
                    junk = pscr.tile([128, D], bf16, tag="junk")
                    nc.scalar.activation(
                        junk[:], prsb[:, D * j:D * (j + 1)], AF.Copy,
                        accum_out=d6b[:, j:j + 1])
                sm6b = psmall.tile([128, 6], f32, tag="sm6b")
                nc.vector.tensor_tensor(sm6b[:], d6b[:], btab[:, 16:22],
                                        OP.mult)
                mb = psmall.tile([128, 1], f32, tag="mb")
                nc.vector.tensor_reduce(mb[:], sm6b[:], mybir.AxisListType.X,
                                        OP.add)
                nc.vector.tensor_tensor(mb[:], mb[:], btab[:, 11:12], OP.mult)
                nc.vector.tensor_scalar(mb[:], mb[:], -2.0 * IV2, None, OP.mult)

                eq16 = psmall.tile([128, 16], f32, tag="eq16")
                acam = atab_sb[:, 24 * mt:24 * mt + 4].rearrange(
                    "p (c o) -> p c o", o=1)
                bcam = btab[:, 0:4].rearrange("p (o c) -> p o c", o=1)
                acb, bcb = bass.broadcast_tensor_aps(acam, bcam)
                nc.vector.tensor_tensor(eq16[:].rearrange("p (i j) -> p i j", j=4),
                                        acb, bcb, OP.is_equal)
                scr16 = psmall.tile([128, 16], f32, tag="scr16")
                sumeq = psmall.tile([128, 1], f32, tag="sumeq")
                nc.vector.tensor_tensor(scr16[:], pab[:], eq16[:], OP.mult)
                nc.vector.tensor_reduce(sumeq[:], scr16[:], mybir.AxisListType.X,
                                        OP.add)
                sumall = psmall.tile([128, 1], f32, tag="sumall")
                nc.vector.tensor_reduce(sumall[:], pab[:], mybir.AxisListType.X,
                                        OP.add)
                scr6 = psmall.tile([128, 6], f32, tag="scr6")
                cntdot = psmall.tile([128, 1], f32, tag="cntdot")
                nc.vector.tensor_tensor(scr6[:], atab_sb[:, 24 * mt + 4:24 * mt + 10],
                                        btab[:, 4:10], OP.mult)
                nc.vector.tensor_reduce(cntdot[:], scr6[:], mybir.AxisListType.X,
                                        OP.add)
                c2 = psmall.tile([128, 1], f32, tag="c2")
                nc.vector.tensor_scalar(c2[:], cntdot[:], -2.0, 32.0,
                                        OP.mult, OP.add)
                c2m = psmall.tile([128, 1], f32, tag="c2m")
                nc.vector.tensor_scalar(c2m[:], c2[:], 1.0, None, OP.max)
                rec2 = psmall.tile([128, 1], f32, tag="rec2")
                nc.vector.reciprocal(rec2[:], c2m[:])
                valid2 = psmall.tile([128, 1], f32, tag="valid2")
                nc.vector.tensor_scalar(valid2[:], c2[:], 0.0, None, OP.is_gt)
                scross = psmall.tile([128, 1], f32, tag="scross")
                nc.vector.tensor_tensor(scross[:], sumall[:], sumeq[:], OP.subtract)
                s2 = psmall.tile([128, 1], f32, tag="s2")
                nc.vector.tensor_scalar(s2[:], scross[:], -2.0 * IV2, None, OP.mult)
                m2 = psmall.tile([128, 1], f32, tag="m2")
                nc.vector.tensor_tensor(m2[:], s2[:], rec2[:], OP.mult)
                va = atab_sb[:, 24 * mt + 10:24 * mt + 11]
                vb = btab[:, 10:11]
                wa = psmall.tile([128, 1], f32, tag="wa")
                nc.vector.tensor_scalar(wa[:], vb, -0.5, 1.0, OP.mult, OP.add)
                nc.vector.tensor_tensor(wa[:], wa[:], va, OP.mult)
                wb = psmall.tile([128, 1], f32, tag="wb")
                nc.vector.tensor_scalar(wb[:], va, -0.5, 1.0, OP.mult, OP.add)
                nc.vector.tensor_tensor(wb[:], wb[:], vb, OP.mult)
                m1a = psmall.tile([128, 1], f32, tag="m1a")
                nc.vector.tensor_tensor(m1a[:], wa[:], ma_list[mt][:], OP.mult)
                m1b = psmall.tile([128, 1], f32, tag="m1b")
                nc.vector.tensor_tensor(m1b[:], wb[:], mb[:], OP.mult)
                m1 = psmall.tile([128, 1], f32, tag="m1")
                nc.vector.tensor_tensor(m1[:], m1a[:], m1b[:], OP.add)
                diff = psmall.tile([128, 1], f32, tag="diff")
                nc.vector.tensor_tensor(diff[:], m1[:], m2[:], OP.subtract)
                nc.vector.tensor_scalar(diff[:], diff[:], MARGIN, None, OP.add)
                lossv = psmall.tile([128, 1], f32, tag="lossv")
                nc.scalar.activation(lossv[:], diff[:], AF.Relu)
                vor = psmall.tile([128, 1], f32, tag="vor")
                nc.vector.tensor_tensor(vor[:], va, vb, OP.mult)
                vsum = psmall.tile([128, 1], f32, tag="vsum")
                nc.vector.tensor_tensor(vsum[:], va, vb, OP.add)
                nc.vector.tensor_tensor(vor[:], vsum[:], vor[:], OP.subtract)
                nc.vector.tensor_tensor(lossv[:], lossv[:], vor[:], OP.mult)
                nc.vector.tensor_tensor(lossv[:], lossv[:], valid2[:], OP.mult)
                nc.tensor.matmul(ploss[:], lhsT=ones_sb[:], rhs=lossv[:],
                                 start=(mt == 0), stop=(mt == PT - 1))

            lsb = psmall.tile([1, 1], f32, tag="lsb")
            nc.scalar.copy(lsb[:], ploss[:])
            nc.sync.dma_start(loss_out[:], lsb[:])

    nc.compile()
    return nc


def _host_prep(input, target, camera_id):
    x = np.ascontiguousarray(np.asarray(input, dtype=np.float32))
    tgt = np.asarray(target).reshape(NG, 4)
    cam = np.asarray(camera_id).reshape(NG, 4)
    labels = tgt[:, 0].astype(np.int64)

    cnt = np.zeros((NG, 6), np.float32)
    for c in range(6):
        cnt[:, c] = (cam == c).sum(axis=1)
    c1 = 16.0 - (cnt * cnt).sum(axis=1)
    rec1 = 1.0 / np.maximum(c1, 1.0)
    va = (c1 > 0).astype(np.float32)

    gtab = np.zeros((NG, 24), np.float32)
    gtab[:, 0:4] = cam.astype(np.float32)
    gtab[:, 4:10] = cnt
    gtab[:, 10] = va
    gtab[:, 11] = rec1
    pairs = [(0, 1), (0, 2), (0, 3), (1, 2), (1, 3), (2, 3)]
    for k, (i, j) in enumerate(pairs):
        gtab[:, 16 + k] = (cam[:, i] != cam[:, j]).astype(np.float32)

    lab16 = labels.astype(np.uint16)
    lab_bc = np.ascontiguousarray(np.broadcast_to(lab16[None, :], (128, NG)))

    ones1 = np.ones((128, 1), np.float32)
    ident = np.eye(128, dtype=ml_dtypes.bfloat16)

    in_maps = []
    for k in range(NCORES):
        g0 = k * G
        in_maps.append({
            "x_sh": x[k * R:(k + 1) * R],
            "lab_bc": lab_bc,
            "lab_loc": np.ascontiguousarray(
                labels[g0:g0 + G].reshape(G, 1).astype(np.float32)),
            "gtab": gtab,
            "atab": np.ascontiguousarray(gtab[g0:g0 + G]),
            "ones1": ones1,
            "ident": ident,
        })
    return in_maps


def kernel(input, target, camera_id):
    if "nc" not in _CACHE:
        _CACHE["nc"] = _build()
    nc = _CACHE["nc"]
    in_maps = _host_prep(input, target, camera_id)
    res = run_bass_kernel_spmd(nc, in_maps, core_ids=list(range(NCORES)))
    total = np.float64(0.0)
    for r in range(NCORES):
        total += np.float64(res.results[r]["loss_part"][0, 0])
    return np.float32(total)
